# revision 1
# baseline (speedup 1.0000x reference)
"""Trainium2 Bass kernel for nn_Decoder (gnn_message_passing), v2.

12-step LSTM decoder with (N,N) pairwise pooling, N=512 agents, sharded over
8 NeuronCores by agent rows (64 agents/core).

Key algebra: the pairwise MLP first layer collapses:
  feat[i,j] = [corr@W_se | h[j] | h[i]],  corr[i,j] = pos[i]-pos[j]
  feat @ W1 + b1 = P[i] + Q[j]
  P[i] = pos[i]@A + h[i]@W1[40:48] + (b1 + b_se@W1[0:32])
  Q[j] = h[j]@W1[32:40] - pos[j]@A,   A = W_se @ W1[0:32]   (64-dim)
Per step each core computes P (own 64 agents) and its Q-block, all-gathers
Q (the only cross-core exchange), then for each pair (i,j):
  ph = relu( relu(P[i]+Q[j]) @ W2 + b2 );  ctx[i] = masked-max_j ph

v2 layout: pool PSUM tile t (agents il = 4*a3 + t, a3 = 4b+2q+p) has
partition u = 16f + a3, built from 8 full-width [128,128] matmuls (FWL).
The neighbor mask folds into the reduce via tensor_tensor_reduce:
  acc[u] = max_j min(psum[u,j], +/-BIG mask)  then  ctx = relu(acc + b2).
ctx transposes to [8,64] with ONE sbuf-to-sbuf DMA (rearrange "(f a) t").
All small matmuls run in bf16; h-dependent/step-invariant partial matmuls
pre-issue before the AllGather so the PE works during the collective.
"""
import numpy as np
import ml_dtypes
from contextlib import ExitStack

import concourse.bass as bass
import concourse.bacc as bacc
import concourse.mybir as mybir
from concourse import tile
from concourse.bass_utils import run_bass_kernel_spmd

F32 = mybir.dt.float32
BF16 = mybir.dt.bfloat16
I32 = mybir.dt.int32

N = 512
R = 8            # cores
NA = N // R      # agents per core = 64
NSTEPS = 12
D = 64           # pooling hidden dim
BIG = 512.0

AluOp = mybir.AluOpType
Act = mybir.ActivationFunctionType


# ---------------------------------------------------------------------------
# host-side constant packing
# ---------------------------------------------------------------------------

def build_constants(W_in, b_in, W_ih, W_hh, b_ih, b_hh, W_m, b_m, W_v, b_v,
                    W_zh, b_zh, W_se, b_se, W1, b1, W2, b2):
    c = {}
    A = W_se @ W1[0:32]                      # [2, 64]
    c["A_T"] = np.ascontiguousarray(A)
    c["negA_T"] = np.ascontiguousarray(-A)
    c["W1u_T"] = np.ascontiguousarray(W1[32:40])   # [8, 64] lhsT for Q (hj)
    c["W1v_T"] = np.ascontiguousarray(W1[40:48])   # [8, 64] lhsT for P (hi)
    c["b1p"] = (b1 + b_se @ W1[0:32]).reshape(64, 1).astype(np.float32)

    # pooling lhsT per (b,q): [128, 128], rows k=64p+d, col u=16f+(4b+2q+p)
    for b in range(4):
        for q in range(2):
            L = np.zeros((128, 128), dtype=np.float32)
            for p in range(2):
                for f in range(8):
                    L[64*p:64*p+64, 16*f + 4*b + 2*q + p] = W2[:, f]
            c[f"Wpool_{b}{q}"] = L

    # mask lhsT [16, 128]: row a3 -> BIG at cols u = 16f + a3
    LM = np.zeros((16, 128), dtype=np.float32)
    for a3 in range(16):
        for f in range(8):
            LM[a3, 16 * f + a3] = BIG
    c["lhsT_mask"] = LM
    # ctx epilogue bias [128, 1]: b2[f] - BIG at u = 16f + a3
    c["bias_ctx"] = (np.repeat(b2, 16) - BIG).reshape(128, 1).astype(np.float32)

    # x layer: x = relu(concat([ctx, prev, c, z]) @ W_in + b_in)
    c["Win_ctx"] = np.ascontiguousarray(W_in[0:8])    # [8, 16]
    c["Win_prev"] = np.ascontiguousarray(W_in[8:10])  # [2, 16]
    c["Win_c"] = np.ascontiguousarray(W_in[10:18])    # [8, 16]
    c["Win_z"] = np.ascontiguousarray(W_in[18:20])    # [2, 16]
    c["b_in"] = b_in.reshape(16, 1).astype(np.float32)

    # gates psum rows: i 0:8, f 32:40, o 64:72, g 96:104 (torch order i,f,g,o)
    Wih2 = np.zeros((16, 104), dtype=np.float32)
    Whh2 = np.zeros((8, 104), dtype=np.float32)
    bias_ifog = np.zeros((104, 1), dtype=np.float32)
    for dst, src in ((0, 0), (32, 8), (64, 24), (96, 16)):
        Wih2[:, dst:dst+8] = W_ih[:, src:src+8]
        Whh2[:, dst:dst+8] = W_hh[:, src:src+8]
        bias_ifog[dst:dst+8, 0] = (b_ih + b_hh)[src:src+8]
    c["Wih_all"] = Wih2
    c["Whh_all"] = Whh2
    c["bias_ifog"] = bias_ifog

    # heads: ps_mv [34, NA]: rows 0:2 mu, 32:34 logvar
    Wmv_h = np.zeros((8, 34), dtype=np.float32)
    Wmv_h[0:4, 0:2] = W_m[0:4]
    Wmv_h[4:8, 32:34] = W_v[0:4]
    Wmv_ctx = np.zeros((8, 34), dtype=np.float32)
    Wmv_ctx[:, 0:2] = W_m[4:12]
    Wmv_ctx[:, 32:34] = W_v[4:12]
    c["Wmv_h"] = Wmv_h
    c["Wmv_ctx"] = Wmv_ctx
    c["b_m"] = b_m.reshape(2, 1).astype(np.float32)
    c["b_v"] = b_v.reshape(2, 1).astype(np.float32)
    c["half_b_v"] = (0.5 * b_v).reshape(2, 1).astype(np.float32)
    c["neg_half_b_v"] = (-0.5 * b_v).reshape(2, 1).astype(np.float32)

    c["Wzh_T"] = np.ascontiguousarray(W_zh)         # [2, 8]
    c["b_zh"] = b_zh.reshape(8, 1).astype(np.float32)
    return c


# consts loaded into SBUF as bf16 (matmul lhsT) vs f32 (biases)
BF16_CONSTS = {"A_T", "negA_T", "W1u_T", "W1v_T", "Win_ctx", "Win_prev",
               "Win_c", "Win_z", "Wih_all", "Whh_all", "Wmv_h", "Wmv_ctx",
               "Wzh_T", "lhsT_mask"} | {f"Wpool_{b}{q}" for b in range(4) for q in range(2)}

CONST_SHAPES = {
    "A_T": [2, D], "negA_T": [2, D], "W1u_T": [8, D], "W1v_T": [8, D],
    "b1p": [D, 1], "bias_ctx": [128, 1], "lhsT_mask": [16, 128],
    "Win_ctx": [8, 16], "Win_prev": [2, 16], "Win_c": [8, 16],
    "Win_z": [2, 16], "b_in": [16, 1],
    "Wih_all": [16, 104], "Whh_all": [8, 104], "bias_ifog": [104, 1],
    "Wmv_h": [8, 34], "Wmv_ctx": [8, 34],
    "b_m": [2, 1], "b_v": [2, 1], "half_b_v": [2, 1], "neg_half_b_v": [2, 1],
    "Wzh_T": [2, 8], "b_zh": [8, 1],
}
for _b in range(4):
    for _q in range(2):
        CONST_SHAPES[f"Wpool_{_b}{_q}"] = [128, 128]


# ---------------------------------------------------------------------------
# device program
# ---------------------------------------------------------------------------

def build_program(nsteps=NSTEPS, dbg=False):
    nc = bacc.Bacc("TRN2", target_bir_lowering=False, debug=False,
                   num_devices=R)

    io = {}
    # per-core inputs
    io["pT"] = nc.dram_tensor("pT", [2, NA], F32, kind="ExternalInput")
    io["cT"] = nc.dram_tensor("cT", [8, NA], F32, kind="ExternalInput")
    io["zT"] = nc.dram_tensor("zT", [2, NA], F32, kind="ExternalInput")
    io["obslastT"] = nc.dram_tensor("obslastT", [2, NA], F32, kind="ExternalInput")
    io["c0T"] = nc.dram_tensor("c0T", [8, NA], F32, kind="ExternalInput")
    io["epsT"] = nc.dram_tensor("epsT", [2, nsteps * NA], F32, kind="ExternalInput")
    io["nei_own"] = nc.dram_tensor("nei_own", [nsteps, NA, N], I32,
                                   kind="ExternalInput")
    for name, shp in CONST_SHAPES.items():
        io[name] = nc.dram_tensor(name, shp, F32, kind="ExternalInput")

    # outputs [2, nsteps, NA]
    o_pos = nc.dram_tensor("out_positions", [2, nsteps, NA], F32, kind="ExternalOutput")
    o_mu = nc.dram_tensor("out_means", [2, nsteps, NA], F32, kind="ExternalOutput")
    o_lv = nc.dram_tensor("out_logvars", [2, nsteps, NA], F32, kind="ExternalOutput")
    if dbg:
        dbg_t = {}
        dbg_t["hT_all"] = nc.dram_tensor("dbg_hT_all", [8, nsteps * NA], F32,
                                         kind="ExternalOutput")
        dbg_t["ctxT_all"] = nc.dram_tensor("dbg_ctxT_all", [8, nsteps * NA],
                                           BF16, kind="ExternalOutput")
        dbg_t["ctx_all"] = nc.dram_tensor("dbg_ctx_all", [128, 3 * 4], BF16,
                                          kind="ExternalOutput")
        dbg_t["mask6"] = nc.dram_tensor("dbg_mask6", [128, N], BF16,
                                        kind="ExternalOutput")
        for nm, shp in (("hT", [8, NA]), ("xT", [16, NA]), ("qblk", [D, NA]),
                        ("PT", [D, NA]), ("pdup", [128, 32]),
                        ("qdup", [128, N]), ("ctx_mx", [128, 4]),
                        ("ctxT", [8, NA]), ("mask0", [128, N]),
                        ("sig_i", [8, NA]), ("tan_g", [8, NA]),
                        ("clT", [8, NA])):
            dbg_t[nm] = nc.dram_tensor(f"dbg_{nm}", shp, F32,
                                       kind="ExternalOutput")

    with tile.TileContext(nc) as tc, ExitStack() as ctx:
        sb1 = ctx.enter_context(tc.tile_pool(name="consts", bufs=1))
        sbs = ctx.enter_context(tc.tile_pool(name="state", bufs=2))
        sbw = ctx.enter_context(tc.tile_pool(name="work", bufs=3))
        sbh = ctx.enter_context(tc.tile_pool(name="h1p", bufs=6))
        sbm = ctx.enter_context(tc.tile_pool(name="maskp", bufs=2))
        sbt = ctx.enter_context(tc.tile_pool(name="ttrscr", bufs=2))
        pp = ctx.enter_context(tc.tile_pool(name="poolps", bufs=4, space="PSUM"))
        sp = ctx.enter_context(tc.tile_pool(name="smallps", bufs=1, space="PSUM"))
        dr = ctx.enter_context(tc.tile_pool(name="dram", bufs=2, space="DRAM"))

        # ---- load constants / inputs into SBUF ----
        cst = {}
        for name, shp in CONST_SHAPES.items():
            if name in BF16_CONSTS:
                t_ = sb1.tile(shp, BF16, tag=name)
                nc.gpsimd.dma_start(t_[:, :], io[name][:, :])
            else:
                t_ = sb1.tile(shp, F32, tag=name)
                nc.sync.dma_start(t_[:, :], io[name][:, :])
            cst[name] = t_

        cT = sb1.tile([8, NA], BF16, tag="cT")
        nc.gpsimd.dma_start(cT[:, :], io["cT"][:, :])
        zT = sb1.tile([2, NA], BF16, tag="zT")
        nc.gpsimd.dma_start(zT[:, :], io["zT"][:, :])
        epsT = sb1.tile([2, nsteps * NA], F32, tag="epsT")
        nc.sync.dma_start(epsT[:, :], io["epsT"][:, :])

        # persistent f32 pos state + bf16 copy for matmuls
        posT = sb1.tile([2, NA], F32, tag="posT")
        nc.sync.dma_start(posT[:, :], io["obslastT"][:, :])
        posTb = sbs.tile([2, NA], BF16, tag="posTb")
        nc.gpsimd.dma_start(posTb[:, :], io["obslastT"][:, :])
        prevb = sbs.tile([2, NA], BF16, tag="prevb")
        nc.gpsimd.dma_start(prevb[:, :], io["pT"][:, :])

        clT = sbs.tile([8, NA], F32, tag="clT")
        nc.sync.dma_start(clT[:, :], io["c0T"][:, :])

        ctxT = sbs.tile([8, NA], BF16, tag="ctxT")
        nc.vector.memset(ctxT[:, :], 0.0)

        # PSUM banks: start=True clears the whole bank, so each bank gets
        # exactly ONE start=True per write-cycle, and that clearing MM is
        # gated (via the hT dependency chain, which is FIFO-after all prior
        # Act/DVE reads) so it cannot race readers of the bank's old data.
        #   bankQ: ps_qp [0:64, 0:128] + h0 [0:8, 384:448], cleared by the
        #          W1u@hT matmul each step
        #   bankP0/P1: step-parity trio: ps_x [0:16, 0:64],
        #          ps_g [0:104, 64:128], ps_mv [0:34, 256:320], cleared by
        #          the first pre-issue matmul in section G
        bankQ = sp.tile([128, 512], F32, tag="bankQ")
        bankP0 = sp.tile([128, 512], F32, tag="bankP0")
        bankP1 = sp.tile([128, 512], F32, tag="bankP1")
        banks = [bankP0, bankP1]

        # h0 = z @ W_zh + b_zh   (first MM into bankQ: start=True)
        nc.tensor.matmul(bankQ[0:8, 384:448], cst["Wzh_T"][:, :], zT[:, :],
                         start=True, stop=True, skip_group_check=True)
        hT = sbs.tile([8, NA], BF16, tag="hT")
        nc.scalar.activation(hT[:, :], bankQ[0:8, 384:448], Act.Identity,
                             bias=cst["b_zh"][:, :])

        # output accumulators [2, nsteps*NA]
        ob_pos = sb1.tile([2, nsteps * NA], F32, tag="ob_pos")
        ob_mu = sb1.tile([2, nsteps * NA], F32, tag="ob_mu")
        ob_lv = sb1.tile([2, nsteps * NA], F32, tag="ob_lv")

        # pre-issue step-0 partials into bankP0 (Whh clears rows 0:104)
        ps_g = bankP0[0:104, 64:128]
        nc.tensor.matmul(ps_g, cst["Whh_all"][:, :], hT[:, :],
                         start=True, stop=False, skip_group_check=True)
        ps_x = bankP0[0:16, 0:64]
        nc.tensor.matmul(ps_x, cst["Win_c"][:, :], cT[:, :],
                         start=False, stop=False, skip_group_check=True)
        nc.tensor.matmul(ps_x, cst["Win_z"][:, :], zT[:, :],
                         start=False, stop=False, skip_group_check=True)
        ps_mv = None   # region, set per step


        def heads_chain(r, ps_mv_r, ctxT_r):
            """heads for step r (after ctx_r known); updates pos state."""
            sl = slice(r * NA, (r + 1) * NA)
            nc.tensor.matmul(ps_mv_r, cst["Wmv_ctx"][:, :], ctxT_r[:, :],
                             start=False, stop=True, skip_group_check=True)
            sp_ = sbw.tile([2, NA], F32, tag="sp")
            nc.scalar.activation(sp_[:, :], ps_mv_r[32:34, 0:64], Act.Sigmoid,
                                 bias=cst["half_b_v"][:, :], scale=0.5)
            sn_ = sbw.tile([2, NA], F32, tag="sn")
            nc.scalar.activation(sn_[:, :], ps_mv_r[32:34, 0:64], Act.Sigmoid,
                                 bias=cst["neg_half_b_v"][:, :], scale=-0.5)
            nc.vector.tensor_scalar_add(ob_mu[:, sl], ps_mv_r[0:2, 0:64],
                                        cst["b_m"][:, :])
            nc.vector.tensor_scalar_add(ob_lv[:, sl], ps_mv_r[32:34, 0:64],
                                        cst["b_v"][:, :])
            rcp = sbw.tile([2, NA], F32, tag="rcp")
            nc.vector.reciprocal(rcp[:, :], sn_[:, :])
            ev = sbw.tile([2, NA], F32, tag="ev")
            nc.vector.tensor_tensor(ev[:, :], sp_[:, :], rcp[:, :],
                                    op=AluOp.mult)
            pe = sbw.tile([2, NA], F32, tag="pe")
            nc.vector.tensor_tensor(pe[:, :], ev[:, :], epsT[:, sl],
                                    op=AluOp.mult)
            nc.vector.tensor_tensor(ob_pos[:, sl], ob_mu[:, sl], pe[:, :],
                                    op=AluOp.add)
            nonlocal posT, posTb, prevb
            posT_new = sbs.tile([2, NA], F32, tag="posT_s")
            nc.vector.tensor_tensor(posT_new[:, :], posT[:, :], ob_pos[:, sl],
                                    op=AluOp.add)
            posT = posT_new
            prevb = sbs.tile([2, NA], BF16, tag="prevb")
            nc.vector.tensor_scalar_add(prevb[:, :], ob_pos[:, sl], 0.0)
            posTb = sbs.tile([2, NA], BF16, tag="posTb")
            nc.scalar.copy(posTb[:, :], posT[:, :])

        for s in range(nsteps):
            # ---------------- heads of previous step ----------------
            masks = [sbm.tile([16, N], BF16, tag=f"mask{t}",
                              name=f"mask{t}") for t in range(4)]
            for t in range(4):
                nc.gpsimd.dma_start(masks[t][:, :], io["nei_own"][s, t::4, :])
            if s > 0:
                heads_chain(s - 1, ps_mv, ctxT)

            # ---------------- x / gates / LSTM ----------------
            nc.tensor.matmul(ps_x, cst["Win_ctx"][:, :], ctxT[:, :],
                             start=False, stop=False, skip_group_check=True)
            nc.tensor.matmul(ps_x, cst["Win_prev"][:, :], prevb[:, :],
                             start=False, stop=True, skip_group_check=True)
            xT = sbw.tile([16, NA], BF16, tag="xT")
            nc.scalar.activation(xT[:, :], ps_x, Act.Relu,
                                 bias=cst["b_in"][:, :])

            nc.tensor.matmul(ps_g, cst["Wih_all"][:, :], xT[:, :],
                             start=False, stop=True, skip_group_check=True)
            sig_i = sbw.tile([8, NA], F32, tag="sig_i")
            nc.scalar.activation(sig_i[:, :], ps_g[0:8, 0:64], Act.Sigmoid,
                                 bias=cst["bias_ifog"][0:8, :])
            sig_f = sbw.tile([8, NA], F32, tag="sig_f")
            nc.scalar.activation(sig_f[:, :], ps_g[32:40, 0:64], Act.Sigmoid,
                                 bias=cst["bias_ifog"][32:40, :])
            sig_o = sbw.tile([8, NA], F32, tag="sig_o")
            nc.scalar.activation(sig_o[:, :], ps_g[64:72, 0:64], Act.Sigmoid,
                                 bias=cst["bias_ifog"][64:72, :])
            tan_g = sbw.tile([8, NA], F32, tag="tan_g")
            nc.scalar.activation(tan_g[:, :], ps_g[96:104, 0:64], Act.Tanh,
                                 bias=cst["bias_ifog"][96:104, :])
            t1 = sbw.tile([8, NA], F32, tag="t1")
            nc.vector.tensor_tensor(t1[:, :], sig_i[:, :], tan_g[:, :],
                                    op=AluOp.mult)
            t2 = sbw.tile([8, NA], F32, tag="t2")
            nc.vector.tensor_tensor(t2[:, :], sig_f[:, :], clT[:, :],
                                    op=AluOp.mult)
            clT = sbs.tile([8, NA], F32, tag="clT")
            nc.vector.tensor_tensor(clT[:, :], t1[:, :], t2[:, :],
                                    op=AluOp.add)
            tcl = sbw.tile([8, NA], F32, tag="tcl")
            nc.scalar.activation(tcl[:, :], clT[:, :], Act.Tanh)
            hT = sbs.tile([8, NA], BF16, tag="hT")
            nc.vector.tensor_tensor(hT[:, :], sig_o[:, :], tcl[:, :],
                                    op=AluOp.mult)
            if dbg and s == 0:
                for nm, tl in (("hT", hT), ("xT", xT), ("sig_i", sig_i),
                               ("tan_g", tan_g), ("clT", clT)):
                    nc.gpsimd.dma_start(dbg_t[nm][:, :], tl[:, :])

            # ---------------- Q + AllGather ----------------
            nxt = banks[(s + 1) % 2]
            ps_qp = bankQ[0:64, 0:128]
            # W1u@hT first: start=True clears bankQ, and its wait on hT
            # orders it after every prior Act/DVE instruction (FIFO), so the
            # clear cannot race reads of last step's Q/P psum
            nc.tensor.matmul(ps_qp[:, 0:NA], cst["W1u_T"][:, :], hT[:, :],
                             start=True, stop=False, skip_group_check=True)
            nc.tensor.matmul(ps_qp[:, 0:NA], cst["negA_T"][:, :], posTb[:, :],
                             start=False, stop=True, skip_group_check=True)
            qblk = sbw.tile([D, NA], BF16, tag="qblk")
            nc.scalar.copy(qblk[:, :], ps_qp[:, 0:NA])
            ag_in = dr.tile([D, NA], BF16, tag="ag_in")
            nc.sync.dma_start(ag_in[:, :], qblk[:, :])
            ag_out = dr.tile([R * D, NA], BF16, tag="ag_out")
            nc.gpsimd.collective_compute(
                "AllGather", AluOp.bypass,
                replica_groups=[list(range(R))],
                ins=[ag_in[:, :]],
                outs=[ag_out[:, :]],
            )

            # ---------------- P / pdup (runs during AllGather) ----------
            nc.tensor.matmul(ps_qp[:, NA:2 * NA], cst["A_T"][:, :],
                             posTb[:, :], start=False, stop=False,
                             skip_group_check=True)
            nc.tensor.matmul(ps_qp[:, NA:2 * NA], cst["W1v_T"][:, :], hT[:, :],
                             start=False, stop=True, skip_group_check=True)
            PT = sbw.tile([D, NA], F32, tag="PT")
            nc.scalar.activation(PT[:, :], ps_qp[:, NA:2 * NA], Act.Identity,
                                 bias=cst["b1p"][:, :])
            # pdup [128, 32]: col pk = 8b+4q+t; lower = P[:, il(p=0)],
            # upper = P[:, il(p=1)], il = 16b+8q+4p+t
            if dbg and s == 0:
                nc.gpsimd.dma_start(dbg_t["qblk"][:, :], qblk[:, :])
                nc.gpsimd.dma_start(dbg_t["PT"][:, :], PT[:, :])
            pdup = sbw.tile([128, 32], F32, tag="pdup")
            for half in range(2):
                for q in range(2):
                    nc.sync.dma_start(
                        pdup.rearrange("P (b q t) -> P q b t", b=4, q=2, t=4)[
                            half * D:(half + 1) * D, q],
                        PT.rearrange("d (b q pt) -> d q b pt", b=4, q=2, pt=8)[
                            :, q, :, 4 * half: 4 * half + 4],
                    )

            # ---------------- pre-issue next step partials ----------------
            if s + 1 < nsteps:
                # Whh first: start=True with 104 written partitions clears
                # rows 0:104 across the whole bank (covers ps_x and ps_mv)
                ps_g = nxt[0:104, 64:128]
                nc.tensor.matmul(ps_g, cst["Whh_all"][:, :], hT[:, :],
                                 start=True, stop=False, skip_group_check=True)
                ps_x = nxt[0:16, 0:64]
                nc.tensor.matmul(ps_x, cst["Win_c"][:, :], cT[:, :],
                                 start=False, stop=False, skip_group_check=True)
                nc.tensor.matmul(ps_x, cst["Win_z"][:, :], zT[:, :],
                                 start=False, stop=False, skip_group_check=True)
            ps_mv = nxt[0:34, 256:320]
            nc.tensor.matmul(ps_mv, cst["Wmv_h"][:, :], hT[:, :],
                             start=(s + 1 >= nsteps), stop=False,
                             skip_group_check=True)


            # ---------------- qdup (waits on AllGather) ----------------
            # qdup [128, 512]: partition (dup, d), free j = 64*rr + jl
            qdup = sbw.tile([128, N], BF16, tag="qdup")
            nc.sync.dma_start(
                qdup[0:D, :].rearrange("d (rr jl) -> d rr jl", rr=R, jl=NA),
                ag_out.rearrange("(rr d) jl -> d rr jl", rr=R, d=D),
            )
            nc.sync.dma_start(
                qdup[D:2 * D, :].rearrange("d (rr jl) -> d rr jl", rr=R, jl=NA),
                ag_out.rearrange("(rr d) jl -> d rr jl", rr=R, d=D),
            )

            if dbg and s == 0:
                nc.gpsimd.dma_start(dbg_t["pdup"][:, :], pdup[:, :])
                nc.gpsimd.dma_start(dbg_t["qdup"][:, :], qdup[:, :])
            # ---------------- pooling ----------------
            ctx_mx = sbw.tile([128, 4], F32, tag="ctx_mx")
            for t in range(4):
                pt_ = pp.tile([128, N], F32, tag="poolps")
                nc.tensor.matmul(pt_[:, :], cst["lhsT_mask"][:, :],
                                 masks[t][:, :], start=True, stop=False,
                                 skip_group_check=True)
                for b in range(4):
                    for q in range(2):
                        pk = 8 * b + 4 * q + t
                        h1 = sbh.tile([128, N], BF16, tag="h1")
                        if pk % 3 == 2:
                            nc.scalar.activation(
                                h1[:, :], qdup[:, :], Act.Relu,
                                bias=pdup[:, pk:pk + 1])
                        else:
                            nc.vector.tensor_scalar(
                                h1[:, :], qdup[:, :], pdup[:, pk:pk + 1], 0.0,
                                op0=AluOp.add, op1=AluOp.max)
                        nc.tensor.matmul(
                            pt_[:, :], cst[f"Wpool_{b}{q}"][:, :], h1[:, :],
                            start=False, stop=(b == 3 and q == 1),
                            skip_group_check=True)
                nc.vector.tensor_reduce(
                    ctx_mx[:, t:t + 1], pt_[:, :], axis=mybir.AxisListType.X,
                    op=AluOp.max)

            # ---------------- ctx epilogue ----------------
            ctx_all = sbw.tile([128, 4], BF16, tag="ctx_all")
            nc.vector.tensor_scalar(ctx_all[:, :], ctx_mx[:, :],
                                    cst["bias_ctx"][:, :], 0.0,
                                    op0=AluOp.add, op1=AluOp.max)
            ctx_d = dr.tile([128, 4], BF16, tag="ctx_d")
            nc.sync.dma_start(ctx_d[:, :], ctx_all[:, :])
            ctxT = sbs.tile([8, NA], BF16, tag="ctxT")
            nc.sync.dma_start(
                ctxT[:, :].rearrange("f (a t) -> f a t", a=16, t=4),
                ctx_d.rearrange("(f a) t -> f a t", f=8, a=16),
            )
            if dbg and s == 0:
                nc.gpsimd.dma_start(dbg_t["ctx_mx"][:, :], ctx_mx[:, :])
                nc.gpsimd.dma_start(dbg_t["ctxT"][:, :], ctxT[:, :])
            if dbg and s <= 2:
                nc.sync.dma_start(
                    dbg_t["ctxT_all"][:, s * NA:(s + 1) * NA], ctxT[:, :])
                nc.sync.dma_start(
                    dbg_t["ctx_all"][:, s * 4:(s + 1) * 4], ctx_all[:, :])

        # final heads + output DMAs
        heads_chain(nsteps - 1, ps_mv, ctxT)
        for ob, od in ((ob_pos, o_pos), (ob_mu, o_mu), (ob_lv, o_lv)):
            nc.sync.dma_start(
                od[:, :, :],
                ob.rearrange("k (s il) -> k s il", s=nsteps, il=NA),
            )

    nc.compile()
    return nc


# ---------------------------------------------------------------------------
# host wrapper
# ---------------------------------------------------------------------------

def make_in_maps(inputs, nsteps=NSTEPS):
    inp = {k: np.asarray(v) for k, v in inputs.items()}
    cst = build_constants(
        inp["W_in"], inp["b_in"], inp["W_ih"], inp["W_hh"], inp["b_ih"],
        inp["b_hh"], inp["W_m"], inp["b_m"], inp["W_v"], inp["b_v"],
        inp["W_zh"], inp["b_zh"], inp["W_se"], inp["b_se"], inp["W1"],
        inp["b1"], inp["W2"], inp["b2"])


    in_maps = []
    for r in range(R):
        sl = slice(r * NA, (r + 1) * NA)
        m = dict(cst)
        m["pT"] = np.ascontiguousarray(inp["p"][sl].T)
        m["cT"] = np.ascontiguousarray(inp["c"][sl].T)
        m["zT"] = np.ascontiguousarray(inp["z"][sl].T)
        m["obslastT"] = np.ascontiguousarray(inp["obs_traj_pos"][-1, sl].T)
        m["c0T"] = np.ascontiguousarray(inp["c0_noise"][sl].T)
        m["epsT"] = np.ascontiguousarray(
            inp["eps"][:nsteps, sl, :].transpose(2, 0, 1).reshape(2, nsteps * NA))
        m["nei_own"] = np.ascontiguousarray(inp["nei_index"][:nsteps, sl, :])
        in_maps.append(m)
    return in_maps


_cached = {}


def kernel(**inputs):
    nsteps = NSTEPS
    if "nc" not in _cached:
        _cached["nc"] = build_program(nsteps)
    nc = _cached["nc"]
    in_maps = make_in_maps(inputs, nsteps)
    res = run_bass_kernel_spmd(nc, in_maps, list(range(R)))
    outs = res.results

    def unshard(name):
        per = [np.asarray(outs[r][name]).transpose(1, 2, 0) for r in range(R)]
        return np.concatenate(per, axis=1)

    return unshard("out_positions"), unshard("out_means"), unshard("out_logvars")



# revision 3
# speedup vs baseline: 1.1943x; 1.1943x over previous
"""Trainium2 Bass kernel for nn_Decoder (gnn_message_passing), v2.

12-step LSTM decoder with (N,N) pairwise pooling, N=512 agents, sharded over
8 NeuronCores by agent rows (64 agents/core).

Key algebra: the pairwise MLP first layer collapses:
  feat[i,j] = [corr@W_se | h[j] | h[i]],  corr[i,j] = pos[i]-pos[j]
  feat @ W1 + b1 = P[i] + Q[j]
  P[i] = pos[i]@A + h[i]@W1[40:48] + (b1 + b_se@W1[0:32])
  Q[j] = h[j]@W1[32:40] - pos[j]@A,   A = W_se @ W1[0:32]   (64-dim)
Per step each core computes P (own 64 agents) and its Q-block, all-gathers
Q (the only cross-core exchange), then for each pair (i,j):
  ph = relu( relu(P[i]+Q[j]) @ W2 + b2 );  ctx[i] = masked-max_j ph

v2 layout: pool PSUM tile t (agents il = 4*a3 + t, a3 = 4b+2q+p) has
partition u = 16f + a3, built from 8 full-width [128,128] matmuls (FWL).
The neighbor mask folds into the reduce via tensor_tensor_reduce:
  acc[u] = max_j min(psum[u,j], +/-BIG mask)  then  ctx = relu(acc + b2).
ctx transposes to [8,64] with ONE sbuf-to-sbuf DMA (rearrange "(f a) t").
All small matmuls run in bf16; h-dependent/step-invariant partial matmuls
pre-issue before the AllGather so the PE works during the collective.
"""
import numpy as np
import ml_dtypes
from contextlib import ExitStack

import concourse.bass as bass
import concourse.bacc as bacc
import concourse.mybir as mybir
from concourse import tile
from concourse.bass_utils import run_bass_kernel_spmd

F32 = mybir.dt.float32
BF16 = mybir.dt.bfloat16
I32 = mybir.dt.int32

N = 512
R = 8            # cores
NA = N // R      # agents per core = 64
NSTEPS = 12
D = 64           # pooling hidden dim
BIG = 512.0

AluOp = mybir.AluOpType
Act = mybir.ActivationFunctionType


# ---------------------------------------------------------------------------
# host-side constant packing
# ---------------------------------------------------------------------------

def build_constants(W_in, b_in, W_ih, W_hh, b_ih, b_hh, W_m, b_m, W_v, b_v,
                    W_zh, b_zh, W_se, b_se, W1, b1, W2, b2):
    c = {}
    A = W_se @ W1[0:32]                      # [2, 64]
    c["A_T"] = np.ascontiguousarray(A)
    c["negA_T"] = np.ascontiguousarray(-A)
    c["W1u_T"] = np.ascontiguousarray(W1[32:40])   # [8, 64] lhsT for Q (hj)
    c["W1v_T"] = np.ascontiguousarray(W1[40:48])   # [8, 64] lhsT for P (hi)
    c["b1p"] = (b1 + b_se @ W1[0:32]).reshape(64, 1).astype(np.float32)

    # pooling lhsT per (b,q): [128, 128], rows k=64p+d, col u=16f+(4b+2q+p)
    for b in range(4):
        for q in range(2):
            L = np.zeros((128, 128), dtype=np.float32)
            for p in range(2):
                for f in range(8):
                    L[64*p:64*p+64, 16*f + 4*b + 2*q + p] = W2[:, f]
            c[f"Wpool_{b}{q}"] = L

    # mask lhsT [16, 128]: row a3 -> BIG at cols u = 16f + a3
    LM = np.zeros((16, 128), dtype=np.float32)
    for a3 in range(16):
        for f in range(8):
            LM[a3, 16 * f + a3] = BIG
    c["lhsT_mask"] = LM
    # ctx epilogue bias [128, 1]: b2[f] - BIG at u = 16f + a3
    c["bias_ctx"] = (np.repeat(b2, 16) - BIG).reshape(128, 1).astype(np.float32)

    # x layer: x = relu(concat([ctx, prev, c, z]) @ W_in + b_in)
    c["Win_ctx"] = np.ascontiguousarray(W_in[0:8])    # [8, 16]
    c["Win_prev"] = np.ascontiguousarray(W_in[8:10])  # [2, 16]
    c["Win_c"] = np.ascontiguousarray(W_in[10:18])    # [8, 16]
    c["Win_z"] = np.ascontiguousarray(W_in[18:20])    # [2, 16]
    c["b_in"] = b_in.reshape(16, 1).astype(np.float32)

    # gates psum rows: i 0:8, f 32:40, o 64:72, g 96:104 (torch order i,f,g,o)
    Wih2 = np.zeros((16, 104), dtype=np.float32)
    Whh2 = np.zeros((8, 104), dtype=np.float32)
    bias_ifog = np.zeros((104, 1), dtype=np.float32)
    for dst, src in ((0, 0), (32, 8), (64, 24), (96, 16)):
        Wih2[:, dst:dst+8] = W_ih[:, src:src+8]
        Whh2[:, dst:dst+8] = W_hh[:, src:src+8]
        bias_ifog[dst:dst+8, 0] = (b_ih + b_hh)[src:src+8]
    c["Wih_all"] = Wih2
    c["Whh_all"] = Whh2
    c["bias_ifog"] = bias_ifog

    # heads: ps_mv [34, NA]: rows 0:2 mu, 32:34 logvar
    Wmv_h = np.zeros((8, 34), dtype=np.float32)
    Wmv_h[0:4, 0:2] = W_m[0:4]
    Wmv_h[4:8, 32:34] = W_v[0:4]
    Wmv_ctx = np.zeros((8, 34), dtype=np.float32)
    Wmv_ctx[:, 0:2] = W_m[4:12]
    Wmv_ctx[:, 32:34] = W_v[4:12]
    c["Wmv_h"] = Wmv_h
    c["Wmv_ctx"] = Wmv_ctx
    c["b_m"] = b_m.reshape(2, 1).astype(np.float32)
    c["b_v"] = b_v.reshape(2, 1).astype(np.float32)
    c["half_b_v"] = (0.5 * b_v).reshape(2, 1).astype(np.float32)
    c["neg_half_b_v"] = (-0.5 * b_v).reshape(2, 1).astype(np.float32)

    c["Wzh_T"] = np.ascontiguousarray(W_zh)         # [2, 8]
    c["b_zh"] = b_zh.reshape(8, 1).astype(np.float32)
    return c


# consts loaded into SBUF as bf16 (matmul lhsT) vs f32 (biases)
BF16_CONSTS = {"A_T", "negA_T", "W1u_T", "W1v_T", "Win_ctx", "Win_prev",
               "Win_c", "Win_z", "Wih_all", "Whh_all", "Wmv_h", "Wmv_ctx",
               "Wzh_T", "lhsT_mask"} | {f"Wpool_{b}{q}" for b in range(4) for q in range(2)}

CONST_SHAPES = {
    "A_T": [2, D], "negA_T": [2, D], "W1u_T": [8, D], "W1v_T": [8, D],
    "b1p": [D, 1], "bias_ctx": [128, 1], "lhsT_mask": [16, 128],
    "Win_ctx": [8, 16], "Win_prev": [2, 16], "Win_c": [8, 16],
    "Win_z": [2, 16], "b_in": [16, 1],
    "Wih_all": [16, 104], "Whh_all": [8, 104], "bias_ifog": [104, 1],
    "Wmv_h": [8, 34], "Wmv_ctx": [8, 34],
    "b_m": [2, 1], "b_v": [2, 1], "half_b_v": [2, 1], "neg_half_b_v": [2, 1],
    "Wzh_T": [2, 8], "b_zh": [8, 1],
}
for _b in range(4):
    for _q in range(2):
        CONST_SHAPES[f"Wpool_{_b}{_q}"] = [128, 128]


# ---------------------------------------------------------------------------
# device program
# ---------------------------------------------------------------------------

def build_program(nsteps=NSTEPS, dbg=False):
    nc = bacc.Bacc("TRN2", target_bir_lowering=False, debug=False,
                   num_devices=R)

    io = {}
    # per-core inputs
    io["pT"] = nc.dram_tensor("pT", [2, NA], F32, kind="ExternalInput")
    io["cT"] = nc.dram_tensor("cT", [8, NA], F32, kind="ExternalInput")
    io["zT"] = nc.dram_tensor("zT", [2, NA], F32, kind="ExternalInput")
    io["obslastT"] = nc.dram_tensor("obslastT", [2, NA], F32, kind="ExternalInput")
    io["c0T"] = nc.dram_tensor("c0T", [8, NA], F32, kind="ExternalInput")
    io["epsT"] = nc.dram_tensor("epsT", [2, nsteps * NA], F32, kind="ExternalInput")
    io["nei_own"] = nc.dram_tensor("nei_own", [nsteps, NA, N], I32,
                                   kind="ExternalInput")
    for name, shp in CONST_SHAPES.items():
        io[name] = nc.dram_tensor(name, shp, F32, kind="ExternalInput")

    # outputs [2, nsteps, NA]
    o_pos = nc.dram_tensor("out_positions", [2, nsteps, NA], F32, kind="ExternalOutput")
    o_mu = nc.dram_tensor("out_means", [2, nsteps, NA], F32, kind="ExternalOutput")
    o_lv = nc.dram_tensor("out_logvars", [2, nsteps, NA], F32, kind="ExternalOutput")
    if dbg:
        dbg_t = {}
        dbg_t["hT_all"] = nc.dram_tensor("dbg_hT_all", [8, nsteps * NA], F32,
                                         kind="ExternalOutput")
        dbg_t["ctxT_all"] = nc.dram_tensor("dbg_ctxT_all", [8, nsteps * NA],
                                           BF16, kind="ExternalOutput")
        dbg_t["ctx_all"] = nc.dram_tensor("dbg_ctx_all", [128, 3 * 4], BF16,
                                          kind="ExternalOutput")
        dbg_t["mask6"] = nc.dram_tensor("dbg_mask6", [128, N], BF16,
                                        kind="ExternalOutput")
        for nm, shp in (("hT", [8, NA]), ("xT", [16, NA]), ("qblk", [D, NA]),
                        ("PT", [D, NA]), ("pdup", [128, 32]),
                        ("qdup", [128, N]), ("ctx_mx", [128, 4]),
                        ("ctxT", [8, NA]), ("mask0", [128, N]),
                        ("sig_i", [8, NA]), ("tan_g", [8, NA]),
                        ("clT", [8, NA])):
            dbg_t[nm] = nc.dram_tensor(f"dbg_{nm}", shp, F32,
                                       kind="ExternalOutput")

    with tile.TileContext(nc) as tc, ExitStack() as ctx:
        sb1 = ctx.enter_context(tc.tile_pool(name="consts", bufs=1))
        sbs = ctx.enter_context(tc.tile_pool(name="state", bufs=2))
        sbw = ctx.enter_context(tc.tile_pool(name="work", bufs=3))
        sbh = ctx.enter_context(tc.tile_pool(name="h1p", bufs=6))
        sbm = ctx.enter_context(tc.tile_pool(name="maskp", bufs=2))
        sbt = ctx.enter_context(tc.tile_pool(name="ttrscr", bufs=2))
        pp = ctx.enter_context(tc.tile_pool(name="poolps", bufs=4, space="PSUM"))
        sp = ctx.enter_context(tc.tile_pool(name="smallps", bufs=1, space="PSUM"))
        dr = ctx.enter_context(tc.tile_pool(name="dram", bufs=2, space="DRAM"))

        # ---- load constants / inputs into SBUF ----
        cst = {}
        for name, shp in CONST_SHAPES.items():
            if name in BF16_CONSTS:
                t_ = sb1.tile(shp, BF16, tag=name)
                nc.gpsimd.dma_start(t_[:, :], io[name][:, :])
            else:
                t_ = sb1.tile(shp, F32, tag=name)
                nc.sync.dma_start(t_[:, :], io[name][:, :])
            cst[name] = t_

        cT = sb1.tile([8, NA], BF16, tag="cT")
        nc.gpsimd.dma_start(cT[:, :], io["cT"][:, :])
        zT = sb1.tile([2, NA], BF16, tag="zT")
        nc.gpsimd.dma_start(zT[:, :], io["zT"][:, :])
        epsT = sb1.tile([2, nsteps * NA], F32, tag="epsT")
        nc.sync.dma_start(epsT[:, :], io["epsT"][:, :])

        # persistent f32 pos state + bf16 copy for matmuls
        posT = sb1.tile([2, NA], F32, tag="posT")
        nc.sync.dma_start(posT[:, :], io["obslastT"][:, :])
        posTb = sbs.tile([2, NA], BF16, tag="posTb")
        nc.gpsimd.dma_start(posTb[:, :], io["obslastT"][:, :])
        prevb = sbs.tile([2, NA], BF16, tag="prevb")
        nc.gpsimd.dma_start(prevb[:, :], io["pT"][:, :])

        clT = sbs.tile([8, NA], F32, tag="clT")
        nc.sync.dma_start(clT[:, :], io["c0T"][:, :])

        ctxT = sbs.tile([8, NA], BF16, tag="ctxT")
        nc.vector.memset(ctxT[:, :], 0.0)

        # PSUM banks: start=True clears the whole bank, so each bank gets
        # exactly ONE start=True per write-cycle, and that clearing MM is
        # gated (via the hT dependency chain, which is FIFO-after all prior
        # Act/DVE reads) so it cannot race readers of the bank's old data.
        #   bankQ: ps_qp [0:64, 0:128] + h0 [0:8, 384:448], cleared by the
        #          W1u@hT matmul each step
        #   bankP0/P1: step-parity trio: ps_x [0:16, 0:64],
        #          ps_g [0:104, 64:128], ps_mv [0:34, 256:320], cleared by
        #          the first pre-issue matmul in section G
        bankQ = sp.tile([128, 512], F32, tag="bankQ")
        bankP0 = sp.tile([128, 512], F32, tag="bankP0")
        bankP1 = sp.tile([128, 512], F32, tag="bankP1")
        banks = [bankP0, bankP1]

        # h0 = z @ W_zh + b_zh   (first MM into bankQ: start=True)
        nc.tensor.matmul(bankQ[0:8, 384:448], cst["Wzh_T"][:, :], zT[:, :],
                         start=True, stop=True, skip_group_check=True)
        hT = sbs.tile([8, NA], BF16, tag="hT")
        nc.scalar.activation(hT[:, :], bankQ[0:8, 384:448], Act.Identity,
                             bias=cst["b_zh"][:, :])

        # output accumulators [2, nsteps*NA]
        ob_pos = sb1.tile([2, nsteps * NA], F32, tag="ob_pos")
        ob_mu = sb1.tile([2, nsteps * NA], F32, tag="ob_mu")
        ob_lv = sb1.tile([2, nsteps * NA], F32, tag="ob_lv")

        # pre-issue step-0 partials into bankP0 (Whh clears rows 0:104)
        ps_g = bankP0[0:104, 64:128]
        nc.tensor.matmul(ps_g, cst["Whh_all"][:, :], hT[:, :],
                         start=True, stop=False, skip_group_check=True)
        ps_x = bankP0[0:16, 0:64]
        nc.tensor.matmul(ps_x, cst["Win_c"][:, :], cT[:, :],
                         start=False, stop=False, skip_group_check=True)
        nc.tensor.matmul(ps_x, cst["Win_z"][:, :], zT[:, :],
                         start=False, stop=False, skip_group_check=True)
        ps_mv = None   # region, set per step


        def heads_chain(r, ps_mv_r, ctxT_r):
            """heads for step r (after ctx_r known); updates pos state."""
            sl = slice(r * NA, (r + 1) * NA)
            nc.tensor.matmul(ps_mv_r, cst["Wmv_ctx"][:, :], ctxT_r[:, :],
                             start=False, stop=True, skip_group_check=True)
            sp_ = sbw.tile([2, NA], F32, tag="sp")
            nc.scalar.activation(sp_[:, :], ps_mv_r[32:34, 0:64], Act.Sigmoid,
                                 bias=cst["half_b_v"][:, :], scale=0.5)
            sn_ = sbw.tile([2, NA], F32, tag="sn")
            nc.scalar.activation(sn_[:, :], ps_mv_r[32:34, 0:64], Act.Sigmoid,
                                 bias=cst["neg_half_b_v"][:, :], scale=-0.5)
            nc.vector.tensor_scalar_add(ob_mu[:, sl], ps_mv_r[0:2, 0:64],
                                        cst["b_m"][:, :])
            nc.vector.tensor_scalar_add(ob_lv[:, sl], ps_mv_r[32:34, 0:64],
                                        cst["b_v"][:, :])
            rcp = sbw.tile([2, NA], F32, tag="rcp")
            nc.vector.reciprocal(rcp[:, :], sn_[:, :])
            ev = sbw.tile([2, NA], F32, tag="ev")
            nc.vector.tensor_tensor(ev[:, :], sp_[:, :], rcp[:, :],
                                    op=AluOp.mult)
            pe = sbw.tile([2, NA], F32, tag="pe")
            nc.vector.tensor_tensor(pe[:, :], ev[:, :], epsT[:, sl],
                                    op=AluOp.mult)
            nc.vector.tensor_tensor(ob_pos[:, sl], ob_mu[:, sl], pe[:, :],
                                    op=AluOp.add)
            nonlocal posT, posTb, prevb
            posT_new = sbs.tile([2, NA], F32, tag="posT_s")
            nc.vector.tensor_tensor(posT_new[:, :], posT[:, :], ob_pos[:, sl],
                                    op=AluOp.add)
            posT = posT_new
            prevb = sbs.tile([2, NA], BF16, tag="prevb")
            nc.vector.tensor_scalar_add(prevb[:, :], ob_pos[:, sl], 0.0)
            posTb = sbs.tile([2, NA], BF16, tag="posTb")
            nc.scalar.copy(posTb[:, :], posT[:, :])

        for s in range(nsteps):
            # ---------------- heads of previous step ----------------
            masks = [sbm.tile([16, N], BF16, tag=f"mask{t}",
                              name=f"mask{t}") for t in range(4)]
            for t in range(4):
                nc.gpsimd.dma_start(masks[t][:, :], io["nei_own"][s, t::4, :])
            if s > 0:
                heads_chain(s - 1, ps_mv, ctxT)

            # ---------------- x / gates / LSTM ----------------
            nc.tensor.matmul(ps_x, cst["Win_ctx"][:, :], ctxT[:, :],
                             start=False, stop=False, skip_group_check=True)
            nc.tensor.matmul(ps_x, cst["Win_prev"][:, :], prevb[:, :],
                             start=False, stop=True, skip_group_check=True)
            xT = sbw.tile([16, NA], BF16, tag="xT")
            nc.scalar.activation(xT[:, :], ps_x, Act.Relu,
                                 bias=cst["b_in"][:, :])

            nc.tensor.matmul(ps_g, cst["Wih_all"][:, :], xT[:, :],
                             start=False, stop=True, skip_group_check=True)
            sig_i = sbw.tile([8, NA], F32, tag="sig_i")
            nc.scalar.activation(sig_i[:, :], ps_g[0:8, 0:64], Act.Sigmoid,
                                 bias=cst["bias_ifog"][0:8, :])
            sig_f = sbw.tile([8, NA], F32, tag="sig_f")
            nc.scalar.activation(sig_f[:, :], ps_g[32:40, 0:64], Act.Sigmoid,
                                 bias=cst["bias_ifog"][32:40, :])
            sig_o = sbw.tile([8, NA], F32, tag="sig_o")
            nc.scalar.activation(sig_o[:, :], ps_g[64:72, 0:64], Act.Sigmoid,
                                 bias=cst["bias_ifog"][64:72, :])
            tan_g = sbw.tile([8, NA], F32, tag="tan_g")
            nc.scalar.activation(tan_g[:, :], ps_g[96:104, 0:64], Act.Tanh,
                                 bias=cst["bias_ifog"][96:104, :])
            t1 = sbw.tile([8, NA], F32, tag="t1")
            nc.vector.tensor_tensor(t1[:, :], sig_i[:, :], tan_g[:, :],
                                    op=AluOp.mult)
            t2 = sbw.tile([8, NA], F32, tag="t2")
            nc.vector.tensor_tensor(t2[:, :], sig_f[:, :], clT[:, :],
                                    op=AluOp.mult)
            clT = sbs.tile([8, NA], F32, tag="clT")
            nc.vector.tensor_tensor(clT[:, :], t1[:, :], t2[:, :],
                                    op=AluOp.add)
            tcl = sbw.tile([8, NA], F32, tag="tcl")
            nc.scalar.activation(tcl[:, :], clT[:, :], Act.Tanh)
            hT = sbs.tile([8, NA], BF16, tag="hT")
            nc.vector.tensor_tensor(hT[:, :], sig_o[:, :], tcl[:, :],
                                    op=AluOp.mult)
            if dbg and s == 0:
                for nm, tl in (("hT", hT), ("xT", xT), ("sig_i", sig_i),
                               ("tan_g", tan_g), ("clT", clT)):
                    nc.gpsimd.dma_start(dbg_t[nm][:, :], tl[:, :])

            # ---------------- Q + AllGather ----------------
            nxt = banks[(s + 1) % 2]
            ps_qp = bankQ[0:64, 0:128]
            # W1u@hT first: start=True clears bankQ, and its wait on hT
            # orders it after every prior Act/DVE instruction (FIFO), so the
            # clear cannot race reads of last step's Q/P psum
            nc.tensor.matmul(ps_qp[:, 0:NA], cst["W1u_T"][:, :], hT[:, :],
                             start=True, stop=False, skip_group_check=True)
            nc.tensor.matmul(ps_qp[:, 0:NA], cst["negA_T"][:, :], posTb[:, :],
                             start=False, stop=True, skip_group_check=True)
            qblk = sbw.tile([D, NA], BF16, tag="qblk")
            nc.scalar.copy(qblk[:, :], ps_qp[:, 0:NA])
            ag_in = dr.tile([D, NA], BF16, tag="ag_in")
            nc.sync.dma_start(ag_in[:, :], qblk[:, :])
            ag_out = dr.tile([R * D, NA], BF16, tag="ag_out")
            nc.gpsimd.collective_compute(
                "AllGather", AluOp.bypass,
                replica_groups=[list(range(R))],
                ins=[ag_in[:, :]],
                outs=[ag_out[:, :]],
            )

            # ---------------- P / pdup (runs during AllGather) ----------
            nc.tensor.matmul(ps_qp[:, NA:2 * NA], cst["A_T"][:, :],
                             posTb[:, :], start=False, stop=False,
                             skip_group_check=True)
            nc.tensor.matmul(ps_qp[:, NA:2 * NA], cst["W1v_T"][:, :], hT[:, :],
                             start=False, stop=True, skip_group_check=True)
            PT = sbw.tile([D, NA], F32, tag="PT")
            nc.scalar.activation(PT[:, :], ps_qp[:, NA:2 * NA], Act.Identity,
                                 bias=cst["b1p"][:, :])
            # pdup [128, 32]: col pk = 8b+4q+t; lower = P[:, il(p=0)],
            # upper = P[:, il(p=1)], il = 16b+8q+4p+t
            if dbg and s == 0:
                nc.gpsimd.dma_start(dbg_t["qblk"][:, :], qblk[:, :])
                nc.gpsimd.dma_start(dbg_t["PT"][:, :], PT[:, :])
            pdup = sbw.tile([128, 32], F32, tag="pdup")
            for half in range(2):
                for q in range(2):
                    nc.sync.dma_start(
                        pdup.rearrange("P (b q t) -> P q b t", b=4, q=2, t=4)[
                            half * D:(half + 1) * D, q],
                        PT.rearrange("d (b q pt) -> d q b pt", b=4, q=2, pt=8)[
                            :, q, :, 4 * half: 4 * half + 4],
                    )

            # ---------------- pre-issue next step partials ----------------
            if s + 1 < nsteps:
                # Whh first: start=True with 104 written partitions clears
                # rows 0:104 across the whole bank (covers ps_x and ps_mv)
                ps_g = nxt[0:104, 64:128]
                nc.tensor.matmul(ps_g, cst["Whh_all"][:, :], hT[:, :],
                                 start=True, stop=False, skip_group_check=True)
                ps_x = nxt[0:16, 0:64]
                nc.tensor.matmul(ps_x, cst["Win_c"][:, :], cT[:, :],
                                 start=False, stop=False, skip_group_check=True)
                nc.tensor.matmul(ps_x, cst["Win_z"][:, :], zT[:, :],
                                 start=False, stop=False, skip_group_check=True)
            ps_mv = nxt[0:34, 256:320]
            nc.tensor.matmul(ps_mv, cst["Wmv_h"][:, :], hT[:, :],
                             start=(s + 1 >= nsteps), stop=False,
                             skip_group_check=True)


            # ---------------- qdup (waits on AllGather) ----------------
            # qdup [128, 512]: partition (dup, d), free j = 64*rr + jl
            qdup = sbw.tile([128, N], BF16, tag="qdup")
            nc.sync.dma_start(
                qdup[0:D, :].rearrange("d (rr jl) -> d rr jl", rr=R, jl=NA),
                ag_out.rearrange("(rr d) jl -> d rr jl", rr=R, d=D),
            )
            nc.gpsimd.dma_start(
                qdup[D:2 * D, :].rearrange("d (rr jl) -> d rr jl", rr=R, jl=NA),
                ag_out.rearrange("(rr d) jl -> d rr jl", rr=R, d=D),
            )

            if dbg and s == 0:
                nc.gpsimd.dma_start(dbg_t["pdup"][:, :], pdup[:, :])
                nc.gpsimd.dma_start(dbg_t["qdup"][:, :], qdup[:, :])
            # ---------------- pooling ----------------
            ctx_mx = sbw.tile([128, 4], F32, tag="ctx_mx")
            for t in range(4):
                pt_ = pp.tile([128, N], F32, tag="poolps")
                nc.tensor.matmul(pt_[:, :], cst["lhsT_mask"][:, :],
                                 masks[t][:, :], start=True, stop=False,
                                 skip_group_check=True)
                for b in range(4):
                    for q in range(2):
                        pk = 8 * b + 4 * q + t
                        h1 = sbh.tile([128, N], BF16, tag="h1")
                        if pk % 3 == 2:
                            nc.scalar.activation(
                                h1[:, :], qdup[:, :], Act.Relu,
                                bias=pdup[:, pk:pk + 1])
                        else:
                            nc.vector.tensor_scalar(
                                h1[:, :], qdup[:, :], pdup[:, pk:pk + 1], 0.0,
                                op0=AluOp.add, op1=AluOp.max)
                        nc.tensor.matmul(
                            pt_[:, :], cst[f"Wpool_{b}{q}"][:, :], h1[:, :],
                            start=False, stop=(b == 3 and q == 1),
                            skip_group_check=True)
                nc.vector.tensor_reduce(
                    ctx_mx[:, t:t + 1], pt_[:, :], axis=mybir.AxisListType.X,
                    op=AluOp.max)

            # ---------------- ctx epilogue ----------------
            ctx_all = sbw.tile([128, 4], BF16, tag="ctx_all")
            nc.vector.tensor_scalar(ctx_all[:, :], ctx_mx[:, :],
                                    cst["bias_ctx"][:, :], 0.0,
                                    op0=AluOp.add, op1=AluOp.max)
            ctxT = sbs.tile([8, NA], BF16, tag="ctxT")
            nc.sync.dma_start(
                ctxT[:, :].rearrange("f (a t) -> f a t", a=16, t=4),
                ctx_all.rearrange("(f a) t -> f a t", f=8, a=16),
            )
            if dbg and s == 0:
                nc.gpsimd.dma_start(dbg_t["ctx_mx"][:, :], ctx_mx[:, :])
                nc.gpsimd.dma_start(dbg_t["ctxT"][:, :], ctxT[:, :])
            if dbg and s <= 2:
                nc.sync.dma_start(
                    dbg_t["ctxT_all"][:, s * NA:(s + 1) * NA], ctxT[:, :])
                nc.sync.dma_start(
                    dbg_t["ctx_all"][:, s * 4:(s + 1) * 4], ctx_all[:, :])

        # final heads + output DMAs
        heads_chain(nsteps - 1, ps_mv, ctxT)
        for ob, od in ((ob_pos, o_pos), (ob_mu, o_mu), (ob_lv, o_lv)):
            nc.sync.dma_start(
                od[:, :, :],
                ob.rearrange("k (s il) -> k s il", s=nsteps, il=NA),
            )

    nc.compile()
    return nc


# ---------------------------------------------------------------------------
# host wrapper
# ---------------------------------------------------------------------------

def make_in_maps(inputs, nsteps=NSTEPS):
    inp = {k: np.asarray(v) for k, v in inputs.items()}
    cst = build_constants(
        inp["W_in"], inp["b_in"], inp["W_ih"], inp["W_hh"], inp["b_ih"],
        inp["b_hh"], inp["W_m"], inp["b_m"], inp["W_v"], inp["b_v"],
        inp["W_zh"], inp["b_zh"], inp["W_se"], inp["b_se"], inp["W1"],
        inp["b1"], inp["W2"], inp["b2"])


    in_maps = []
    for r in range(R):
        sl = slice(r * NA, (r + 1) * NA)
        m = dict(cst)
        m["pT"] = np.ascontiguousarray(inp["p"][sl].T)
        m["cT"] = np.ascontiguousarray(inp["c"][sl].T)
        m["zT"] = np.ascontiguousarray(inp["z"][sl].T)
        m["obslastT"] = np.ascontiguousarray(inp["obs_traj_pos"][-1, sl].T)
        m["c0T"] = np.ascontiguousarray(inp["c0_noise"][sl].T)
        m["epsT"] = np.ascontiguousarray(
            inp["eps"][:nsteps, sl, :].transpose(2, 0, 1).reshape(2, nsteps * NA))
        m["nei_own"] = np.ascontiguousarray(inp["nei_index"][:nsteps, sl, :])
        in_maps.append(m)
    return in_maps


_cached = {}


def kernel(**inputs):
    nsteps = NSTEPS
    if "nc" not in _cached:
        _cached["nc"] = build_program(nsteps)
    nc = _cached["nc"]
    in_maps = make_in_maps(inputs, nsteps)
    res = run_bass_kernel_spmd(nc, in_maps, list(range(R)))
    outs = res.results

    def unshard(name):
        per = [np.asarray(outs[r][name]).transpose(1, 2, 0) for r in range(R)]
        return np.concatenate(per, axis=1)

    return unshard("out_positions"), unshard("out_means"), unshard("out_logvars")

